# revision 19
# baseline (speedup 1.0000x reference)
"""Trainium2 Bass kernel for a top-2 gated MoE layer (8 experts, H=1024, F=4096).

Strategy (expert parallelism across the 8 NeuronCores):
  - Host computes the top-2 routing (argsort of the fp32 gate logits) AND the
    combine weights comb = softmax(top2) * alpha, gathers each expert's tokens
    into a padded, transposed activation block xgT [H, C] (C = padded
    per-expert capacity).  All heavy math runs on device; the host only
    shards/gathers.
  - Each core runs one expert: LayerNorm + fc1 + gelu + fc2 + bias + gate
    scaling (matmuls in bf16 with fp32 PSUM accumulation, LN statistics in
    fp32).
  - Host scatter-adds the per-expert outputs back into the full [B,S,H]
    tensor.

Self-contained: shapes are hardcoded from the problem spec.
"""

import numpy as np
import ml_dtypes
from contextlib import ExitStack

TOP_K = 2
LN_EPS = 1e-5
B, S, H, E, F = 2, 2048, 1024, 8, 4096
T = B * S
P = 128
KH = H // P          # 8 H-tiles
FB = 1024            # F block size
NFB = F // FB        # 4 blocks
MF = FB // P         # 8 F-tiles per block

_BUILD_CACHE = {}


def _chunks(C):
    # Small first chunk so the LN -> fc1 pipeline fills quickly.
    out = []
    off = 0
    if C >= 768:
        out.append((0, 256))
        off = 256
    while C - off > 512:
        out.append((off, 512))
        off += 512
    if C - off:
        out.append((off, C - off))
    return out


def _build(C):
    """Build + compile the single-core Bass program (SPMD across 8 cores)."""
    if C in _BUILD_CACHE:
        return _BUILD_CACHE[C]

    import concourse.bass as bass  # noqa: F401
    import concourse.tile as tile
    import concourse.mybir as mybir
    from concourse import bacc, bass_isa

    bf = mybir.dt.bfloat16
    f32 = mybir.dt.float32
    AF = mybir.ActivationFunctionType
    OP = mybir.AluOpType

    nc = bacc.Bacc("TRN2", target_bir_lowering=False, debug=False, num_devices=8)

    d_xgT = nc.dram_tensor("xgT", [H, C], bf, kind="ExternalInput")
    d_w1 = nc.dram_tensor("w1", [H, F], bf, kind="ExternalInput")
    d_w2 = nc.dram_tensor("w2", [F, H], bf, kind="ExternalInput")
    d_b1r = nc.dram_tensor("b1r", [P, F // P], f32, kind="ExternalInput")
    d_pp = nc.dram_tensor("pp", [P, 3 * KH], f32, kind="ExternalInput")
    d_comb = nc.dram_tensor("comb", [1, C], bf, kind="ExternalInput")
    d_y = nc.dram_tensor("ytT", [H, C], bf, kind="ExternalOutput")

    chunks = _chunks(C)

    with tile.TileContext(nc) as tc, ExitStack() as ctx:
        const = ctx.enter_context(tc.tile_pool(name="const", bufs=1))
        gpool = ctx.enter_context(tc.tile_pool(name="gate", bufs=1))
        bpool = ctx.enter_context(tc.tile_pool(name="bcast", bufs=1))
        xpool = ctx.enter_context(tc.tile_pool(name="x", bufs=1))
        w1a = ctx.enter_context(tc.tile_pool(name="w1a", bufs=4))
        w2a = ctx.enter_context(tc.tile_pool(name="w2a", bufs=2))
        sqpool = ctx.enter_context(tc.tile_pool(name="sq", bufs=8))
        tpool = ctx.enter_context(tc.tile_pool(name="t1", bufs=3))
        hpool = ctx.enter_context(tc.tile_pool(name="hdn", bufs=KH))
        w1pool = ctx.enter_context(tc.tile_pool(name="w1", bufs=2))
        w2pool = ctx.enter_context(tc.tile_pool(name="w2", bufs=2))
        apool = ctx.enter_context(tc.tile_pool(name="acts", bufs=8))
        ypool = ctx.enter_context(tc.tile_pool(name="yacc", bufs=1))
        ps_small = ctx.enter_context(
            tc.tile_pool(name="ps_small", bufs=2, space="PSUM"))
        ps1 = ctx.enter_context(tc.tile_pool(name="ps1", bufs=3, space="PSUM"))
        ps2 = ctx.enter_context(tc.tile_pool(name="ps2", bufs=3, space="PSUM"))

        # ---- constants / small params ----
        ones_k = const.tile([P, 1], bf)
        nc.vector.memset(ones_k, 1.0)
        # all-ones [128,128] lhsT: column-reduces AND partition-broadcasts
        # in one matmul (out[m,c] = sum_k x[k,c] for every m)
        ones_mat = const.tile([P, P], bf)
        nc.vector.memset(ones_mat, 1.0)
        glib0 = const.tile([E, 1], f32)
        glib1 = const.tile([E, 1], f32)
        glibs = const.tile([E, 1], f32)
        glibr = const.tile([E, 1], f32)
        nc.vector.memset(glib0, 1.0)
        # pre-warm: gpsimd partition-op library, the Sqrt ACT table, and the
        # custom-DVE reciprocal uops — all during the initial x DMA wait, so
        # none of these one-time costs land on chunk 0's critical path
        nc.gpsimd.partition_broadcast(glib1[:], glib0[0:1, :], E)
        nc.scalar.activation(glibs[:], glib0[:], AF.Sqrt)
        nc.vector.reciprocal_approx_fast(out=glibr[:], in_=glib0[:])
        # PE warm-up: ~2us of junk matmuls trains the HAM clock gate toward
        # 2.4 GHz while the first x DMAs are still in flight.
        warm_rhs = const.tile([P, 512], bf)
        nc.vector.memset(warm_rhs, 0.0)
        ps_w = ps_small.tile([1, 512], f32, tag="pss", name="warm")
        for i in range(12):
            nc.tensor.matmul(ps_w[:], ones_k[:], warm_rhs[:],
                             start=True, stop=True)

        # ---- DMA issue order: x chunk0 -> w1 block0 (split) -> x rest ->
        # w2 block0 (split) -> small params.  fc1 of chunk 0 can start as
        # soon as the first w1 piece + chunk0 stats are in.
        xbig = xpool.tile([P, KH, C], bf, tag="xk", name="xbig")
        xk = [xbig[:, k, :] for k in range(KH)]
        d_xr = d_xgT.ap().rearrange("(k p) c -> p k c", p=P)
        nc.sync.dma_start(xbig[:, :, 0:chunks[0][1]],
                          d_xr[:, :, 0:chunks[0][1]])

        # block-0 weights split along the OUTPUT axis (f for w1, h for w2)
        # into SEPARATE tiles: DMA-completion semaphores are per-tile, so
        # the first fc1/fc2 psum groups start as soon as their piece lands.
        w1p0 = []
        for piece in range(4):
            t = w1a.tile([P, KH, 256], bf, tag="w1a", name=f"w1a_{piece}")
            nc.sync.dma_start(
                t[:],
                d_w1.ap()[:, 256 * piece:256 * piece + 256].rearrange(
                    "(k p) f -> p k f", p=P))
            w1p0.append(t)

        for (off, w) in chunks[1:]:
            nc.sync.dma_start(xbig[:, :, off:off + w], d_xr[:, :, off:off + w])

        w2p0 = []
        for piece in range(2):
            t = w2a.tile([P, MF, 512], bf, tag="w2a", name=f"w2a_{piece}")
            nc.sync.dma_start(
                t[:],
                d_w2.ap()[0:FB, 512 * piece:512 * piece + 512].rearrange(
                    "(k p) h -> p k h", p=P))
            w2p0.append(t)

        pp_sb = const.tile([P, 3 * KH], f32)
        nc.sync.dma_start(pp_sb[:], d_pp.ap())
        lnw_sb = pp_sb[:, 0:KH]
        lnb_sb = pp_sb[:, KH:2 * KH]
        b2_sb = pp_sb[:, 2 * KH:3 * KH]
        b1_sb = const.tile([P, F // P], f32)
        nc.sync.dma_start(b1_sb[:], d_b1r.ap())
        comb_row = gpool.tile([1, C], bf)
        nc.sync.dma_start(comb_row[:], d_comb.ap())
        comb_b = bpool.tile([P, C], bf)
        nc.gpsimd.partition_broadcast(comb_b[:], comb_row[:], P)

        # ---- Phases A-C, pipelined along C-chunks so the PE can start the
        # fc1 matmuls of chunk 0 while later chunks are still in LN ----
        eps_t = gpool.tile([P, 1], f32)
        nc.vector.memset(eps_t, float(LN_EPS))
        # negated ln_w columns: hdn = ((mean_b - x) * -lnw) * inv + lnb
        nlnw_sb = gpool.tile([P, KH], f32)
        nc.scalar.activation(nlnw_sb[:], lnw_sb[:], AF.Identity,
                             bias=0.0, scale=-1.0)
        hdn = [hpool.tile([P, C], bf, tag="hdn", name=f"hdn{k}")
               for k in range(KH)]
        m_b = bpool.tile([P, C], bf)
        inv_b = bpool.tile([P, C], f32)

        sq_t = {}
        ybig = ypool.tile([P, KH, C], bf, tag="yacc", name="ybig")
        y_acc = [ybig[:, h, :] for h in range(KH)]
        d_yr = d_y.ap().rearrange("(k p) c -> p k c", p=P)

        def load_w_block(fb):
            w1blk = w1pool.tile([P, KH, FB], bf, tag="w1", name=f"w1_{fb}")
            nc.sync.dma_start(
                w1blk[:],
                d_w1.ap()[:, fb * FB:(fb + 1) * FB].rearrange(
                    "(k p) f -> p k f", p=P))
            w2blk = w2pool.tile([P, MF, H], bf, tag="w2", name=f"w2_{fb}")
            nc.sync.dma_start(
                w2blk[:],
                d_w2.ap()[fb * FB:(fb + 1) * FB, :].rearrange(
                    "(k p) h -> p k h", p=P))
            return ([w1blk[:, k, :] for k in range(KH)],
                    [w2blk[:, k, :] for k in range(MF)])

        at0 = [apool.tile([P, C], bf, tag="acts", name=f"a_0_{m}")
               for m in range(MF)]

        def emit_prologue(ci):
            off, w = chunks[ci]
            sl = slice(off, off + w)
            # x^2 split between DVE and ACT so neither engine gates the chain
            for k in range(KH):
                sq_c = sqpool.tile([P, w], bf, tag="sq", name=f"sq_{off}_{k}")
                if k < 4:
                    nc.vector.tensor_mul(sq_c[:], xk[k][:, sl], xk[k][:, sl])
                else:
                    nc.scalar.activation(sq_c[:], xk[k][:, sl], AF.Square)
                sq_t[(ci, k)] = sq_c
            # column sums + sums of squares, pre-broadcast to all partitions
            # by the all-ones lhsT (same PE cycles as a 1-row reduction)
            ps_a = ps_small.tile([P, w], f32, tag="pss", name=f"ps_sl{off}")
            for k in range(KH):
                nc.tensor.matmul(ps_a[:], ones_mat[:], xk[k][:, sl],
                                 start=(k == 0), stop=(k == KH - 1))
            nc.scalar.activation(m_b[:, sl], ps_a[:], AF.Identity,
                                 bias=0.0, scale=1.0 / H)
            ps_b = ps_small.tile([P, w], f32, tag="pss", name=f"ps_sq{off}")
            for k in range(KH):
                nc.tensor.matmul(ps_b[:], ones_mat[:], sq_t[(ci, k)][:],
                                 start=(k == 0), stop=(k == KH - 1))

            # var = sumsq/H - mean^2 ; inv = 1/sqrt(var + eps)
            v0 = tpool.tile([P, w], f32, tag="t1", name=f"v0_{off}")
            nc.vector.scalar_tensor_tensor(v0[:], m_b[:, sl], -1.0,
                                           m_b[:, sl], OP.mult, OP.mult)
            nc.vector.scalar_tensor_tensor(v0[:], ps_b[:], 1.0 / H,
                                           v0[:], OP.mult, OP.add)
            nc.scalar.activation(v0[:], v0[:], AF.Sqrt,
                                 bias=eps_t[:], scale=1.0)
            nc.vector.reciprocal_approx_fast(out=inv_b[:, sl], in_=v0[:])

            # apply LayerNorm -> hdn (bf16):
            #   hdn = ((m_b - x) * -lnw) * inv + lnb
            for k in range(KH):
                t1 = tpool.tile([P, w], bf, tag="t1b", name=f"t1_{off}_{k}")
                nc.vector.tensor_sub(t1[:], m_b[:, sl], xk[k][:, sl])
                nc.vector.scalar_tensor_tensor(t1[:], t1[:],
                                               nlnw_sb[:, k:k + 1],
                                               inv_b[:, sl], OP.mult, OP.mult)
                nc.scalar.activation(hdn[k][:, sl], t1[:], AF.Identity,
                                     bias=lnb_sb[:, k:k + 1], scale=1.0)

        def emit_fb0(ci):
            # F-block 0 fc1 -> gelu -> fc2 on this chunk (piece-split weights)
            off, w = chunks[ci]
            sl = slice(off, off + w)
            for m in range(MF):
                w1piece = w1p0[m // 2]
                msl = slice((m % 2) * P, (m % 2) * P + P)
                pst = ps1.tile([P, w], f32, tag="ps1", name=f"ps1_0_{m}_{ci}")
                for k in range(KH):
                    nc.tensor.matmul(pst[:], w1piece[:, k, msl],
                                     hdn[k][:, sl],
                                     start=(k == 0), stop=(k == KH - 1))
                nc.scalar.activation(at0[m][:, sl], pst[:],
                                     AF.Gelu_apprx_tanh,
                                     bias=b1_sb[:, m:m + 1])
            for h in range(KH):
                w2piece = w2p0[h // 4]
                hsl = slice((h % 4) * P, (h % 4) * P + P)
                pst = ps2.tile([P, w], f32, tag="ps2", name=f"ps2_0_{h}_{ci}")
                for k in range(MF):
                    nc.tensor.matmul(pst[:], w2piece[:, k, hsl],
                                     at0[k][:, sl],
                                     start=(k == 0), stop=(k == MF - 1))
                nc.scalar.activation(y_acc[h][:, sl], pst[:], AF.Identity,
                                     bias=0.0)

        # software pipeline: each chunk's stats/LN are emitted one chunk
        # ahead of its block-0 compute, so no engine's (in-order) queue makes
        # chunk c+1's LN wait behind chunk c's fb0-related work.
        emit_prologue(0)
        for ci in range(1, len(chunks)):
            emit_prologue(ci)
            emit_fb0(ci - 1)
        emit_fb0(len(chunks) - 1)

        # ---- Phase D: remaining F blocks.  Middle blocks iterate
        # weight-stationary (each lhsT feeds all chunks); the last block
        # iterates per-chunk so the finalize tail is short. ----
        for fb in range(1, NFB):
            w1t, w2t = load_w_block(fb)

            at = [apool.tile([P, C], bf, tag="acts", name=f"a_{fb}_{m}")
                  for m in range(MF)]
            if fb == NFB - 1:
                # per-range groups, widest first; the final (exposed) tail
                # range is split in half so the finalize+store drain is short
                rs = sorted(chunks, key=lambda c: -c[1])
                last_off, last_w = rs[-1]
                groups = ([[r] for r in rs[:-1]]
                          + [[(last_off, last_w // 2)],
                             [(last_off + last_w // 2, last_w - last_w // 2)]])
            else:
                groups = [list(chunks)]

            for rg in groups:
                for m in range(MF):
                    psg = {r: ps1.tile([P, r[1]], f32, tag="ps1",
                                       name=f"ps1_{fb}_{m}_{r[0]}")
                           for r in rg}
                    for k in range(KH):
                        lhsT = w1t[k][:, m * P:(m + 1) * P]
                        for r in rg:
                            off, w = r
                            nc.tensor.matmul(psg[r][:], lhsT,
                                             hdn[k][:, off:off + w],
                                             start=(k == 0), stop=(k == KH - 1))
                    fcol = fb * MF + m
                    for r in rg:
                        off, w = r
                        nc.scalar.activation(at[m][:, off:off + w], psg[r][:],
                                             AF.Gelu_apprx_tanh,
                                             bias=b1_sb[:, fcol:fcol + 1])
                for h in range(KH):
                    psg = {r: ps2.tile([P, r[1]], f32, tag="ps2",
                                       name=f"ps2_{fb}_{h}_{r[0]}")
                           for r in rg}
                    for k in range(MF):
                        lhsT = w2t[k][:, h * P:(h + 1) * P]
                        for r in rg:
                            off, w = r
                            nc.tensor.matmul(psg[r][:], lhsT,
                                             at[k][:, off:off + w],
                                             start=(k == 0), stop=(k == MF - 1))
                    for r in rg:
                        off, w = r
                        if fb < NFB - 1:
                            nc.vector.tensor_add(y_acc[h][:, off:off + w],
                                                 y_acc[h][:, off:off + w],
                                                 psg[r][:])
                        else:
                            # fused finalize: y = (psum + b2) + y_acc, then
                            # scale by the gate weight and store this chunk
                            nc.vector.scalar_tensor_tensor(
                                y_acc[h][:, off:off + w], psg[r][:],
                                b2_sb[:, h:h + 1], y_acc[h][:, off:off + w],
                                OP.add, OP.add)
                            nc.vector.tensor_mul(y_acc[h][:, off:off + w],
                                                 y_acc[h][:, off:off + w],
                                                 comb_b[:, off:off + w])
                            nc.sync.dma_start(
                                d_yr[:, h:h + 1, off:off + w],
                                ybig[:, h:h + 1, off:off + w])

    nc.compile()
    _BUILD_CACHE[C] = nc
    return nc


def _prepare(x, Wg, alpha, ln_w, ln_b, fc1_w, fc1_b, fc2_w, fc2_b):
    """Host-side routing + per-core input construction."""
    bfnp = ml_dtypes.bfloat16
    xf = np.asarray(x, np.float32).reshape(T, H)
    Wg = np.asarray(Wg, np.float32)
    alpha = np.asarray(alpha, np.float32)
    ln_w = np.asarray(ln_w, np.float32)
    ln_b = np.asarray(ln_b, np.float32)
    fc1_w = np.asarray(fc1_w, np.float32)
    fc1_b = np.asarray(fc1_b, np.float32)
    fc2_w = np.asarray(fc2_w, np.float32)
    fc2_b = np.asarray(fc2_b, np.float32)

    logits = xf @ Wg
    order = np.argsort(-logits, axis=1, kind="stable")
    top2 = order[:, :TOP_K]
    tv = np.take_along_axis(logits, top2, 1)
    sm = np.exp(tv - tv.max(1, keepdims=True))
    sm /= sm.sum(1, keepdims=True)
    comb = np.zeros((T, E), np.float32)
    np.put_along_axis(comb, top2, sm, 1)
    comb *= alpha
    sel = np.zeros((T, E), dtype=bool)
    sel[np.arange(T)[:, None], top2] = True
    idx = [np.nonzero(sel[:, e])[0] for e in range(E)]

    maxc = max(len(i) for i in idx)
    C = max(512, 16 * ((maxc + 15) // 16))

    KHp = H // 128
    in_maps = []
    for e in range(E):
        n = len(idx[e])
        xg = np.zeros((C, H), np.float32)
        xg[:n] = xf[idx[e]]
        cv = np.zeros((1, C), bfnp)
        cv[0, :n] = comb[idx[e], e].astype(bfnp)
        pp = np.concatenate([
            ln_w[e].reshape(KHp, 128).T,
            ln_b[e].reshape(KHp, 128).T,
            fc2_b[e].reshape(KHp, 128).T,
        ], axis=1)
        in_maps.append({
            "xgT": np.ascontiguousarray(xg.T).astype(bfnp),
            "w1": fc1_w[e].astype(bfnp),
            "w2": fc2_w[e].astype(bfnp),
            "b1r": np.ascontiguousarray(fc1_b[e].reshape(F // 128, 128).T),
            "pp": np.ascontiguousarray(pp),
            "comb": cv,
        })
    return in_maps, idx, C


def _kernel_impl(inputs, trace=False, trace_cores=None):
    from concourse import bass_utils

    in_maps, idx, C = _prepare(**inputs)
    nc = _build(C)
    res = bass_utils.run_bass_kernel_spmd(
        nc, in_maps, core_ids=list(range(E)),
        trace=trace, trace_cores=trace_cores)

    out = np.zeros((T, H), np.float32)
    for e in range(E):
        yt = np.asarray(res.results[e]["ytT"], np.float32)  # [H, C]
        n = len(idx[e])
        out[idx[e]] += yt.T[:n]
    return out.reshape(B, S, H), res


def kernel(**inputs):
    out, _ = _kernel_impl(inputs)
    return out
